# revision 17
# baseline (speedup 1.0000x reference)
"""KPConv Bass/Trainium2 kernel (combined-row gather, fp16 compute).

out[m,d] = sum_k ( sum_h infl[m,h,k] * s_feats[idx[m,h],:] ) @ W[k]
infl[m,h,k] = relu(1 - |s_pts[idx[m,h]] - q_pts[m] - kp[k]| / SIGMA)

Sharding: query points M=50000 split 8 ways (6250/core, padded to 6272 =
49 blocks x 128 points). Support data replicated per core as a combined
fp16 row table s_comb[N, 132] = [x, y, z, pad, feat_0..feat_127], so one
gathered 264B row carries both the neighbor's coords and its features
(half the indirect-DMA call count of separate s_pts/s_feats gathers).

Per-core dataflow, per block of 128 query points (= 32 "tiles" of 4
points x 32 neighbors = 128 edges each):
  1. 32 indirect-DMA gathers (one per tile; the HW indirect DMA consumes
     exactly one index per partition, so 128 rows per call is the max)
     into SBUF nf [128, 32*132] fp16 - edge (g,h) of tile t sits in
     partition g*32+h at cols t*132. Coords at +0..2, feats at +4..131.
  2. influence on DVE/ACT in fp16 (2x DVE rate): delta, (delta-kp)^2,
     segmented reduce, sqrt, relu affine, block-diag mask -> bd.
  3. step A on PE (fp16, FWL): per tile t, matmul(lhsT=feats_t
     [128e,128c], rhs=bd_t [128e, 60]) -> PSUM wfT [128c, m*15+k].
  4. step B on PE (fp16): per k, matmul(lhsT=wfT[:, k::15] [c,m],
     rhs=W[k] [c,d]) accumulating over k -> PSUM [128m, 128d] -> DRAM.

Bottleneck: the Pool/GPSIMD engine's SWDGE descriptor generation
(~0.5-1us per indirect DMA x 49*32 calls/core). Batched alternatives
(multi-index indirect DMA, the Ant dma_gather) were probed on HW: the
former silently gathers only the first index per partition, the latter
needs load_library(mlp), which crashes this axon environment
(NRT_EXEC_UNIT_UNRECOVERABLE).
"""

import sys

sys.path.insert(0, "/opt/trn_rl_repo")

import numpy as np

# ---------------------------------------------------------------- constants
N_CORES = 8
M_TOTAL = 50000
N_SUP = 50000
H = 32
C = 128
K = 15
SIGMA = 2.0

M_CORE = M_TOTAL // N_CORES          # 6250
P = 128                              # partitions / points per block
NB = (M_CORE + P - 1) // P           # 49 blocks
M_PAD = NB * P                       # 6272
G = 4                                # points per step-A matmul tile
NT = P // G                          # 32 tiles per block
WROW = 132                           # fp16 elems per s_comb row
CO = 4                               # feat col offset within a row

_compiled = None


def _build_bass(
    nb=NB, n_sup=N_SUP, compile=True, repeats=1, parts="all", gather_mode="tile"
):
    """Build + compile the per-core SPMD Bass program."""
    from contextlib import ExitStack

    import concourse.bacc as bacc
    import concourse.mybir as mybir
    import concourse.tile as tile
    from concourse import bass

    f32 = mybir.dt.float32
    f16 = mybir.dt.float16
    i32 = mybir.dt.int32
    NB = nb
    N_SUP_ = n_sup

    nc = bacc.Bacc(
        "TRN2",
        target_bir_lowering=False,
        debug=False,
        enable_asserts=False,
        num_devices=N_CORES,
    )

    q_blk_d = nc.dram_tensor("q_blk", (NB, P, NT * 3), f16, kind="ExternalInput")
    inds_d = nc.dram_tensor("inds_blk", (NB, P, NT), i32, kind="ExternalInput")
    scomb_d = nc.dram_tensor("s_comb", (N_SUP_, WROW), f16, kind="ExternalInput")
    w_d = nc.dram_tensor("weights", (K, C, C), f16, kind="ExternalInput")
    kp_d = nc.dram_tensor("kp_rep", (P, K * 3), f16, kind="ExternalInput")
    mask_d = nc.dram_tensor("mask60", (P, G * K), f16, kind="ExternalInput")
    out_d = nc.dram_tensor("out", (NB, P, C), f32, kind="ExternalOutput")

    sub = mybir.AluOpType.subtract
    mult = mybir.AluOpType.mult

    with tile.TileContext(nc) as tc, ExitStack() as ctx:
        const = ctx.enter_context(tc.tile_pool(name="const", bufs=1))
        io = ctx.enter_context(tc.tile_pool(name="io", bufs=4))
        mid = ctx.enter_context(tc.tile_pool(name="mid", bufs=3))
        psa = ctx.enter_context(tc.tile_pool(name="psa", bufs=1, space="PSUM"))
        psb = ctx.enter_context(tc.tile_pool(name="psb", bufs=2, space="PSUM"))

        # constants: weights as [c, k, d], kernel points, block-diag mask.
        # On the scalar engine's HWDGE queue so block 0's inds DMA (which
        # gates the first gather) is first in line on the sync queue.
        w_sb = const.tile([P, K, C], f16)
        nc.scalar.dma_start(w_sb[:], w_d.ap().rearrange("k c d -> c k d"))
        kp_sb = const.tile([P, K * 3], f16)
        nc.scalar.dma_start(kp_sb[:], kp_d.ap())
        mask_sb = const.tile([P, G * K], f16)
        nc.scalar.dma_start(mask_sb[:], mask_d.ap())

        do_gather = parts != "compute"
        do_compute = parts not in ("gather", "gather_t")
        for B in [b for _ in range(repeats) for b in range(NB)]:
            # tail block: only ceil((M_CORE - 128*(NB-1)) / G) tiles hold
            # real points; the rest is padding whose output rows are
            # discarded at unshard, so skip their gathers and compute.
            full = B < NB - 1 or nb != (M_CORE + P - 1) // P
            ntb = NT if full else (M_CORE - P * (NB - 1) + G - 1) // G

            inds = io.tile([P, NT], i32, tag="inds")
            nc.sync.dma_start(inds[:], inds_d.ap()[B])
            qb = io.tile([P, NT * 3], f16, tag="qb")
            nc.sync.dma_start(qb[:], q_blk_d.ap()[B])

            # gather the block's neighbor rows, one tile (128 rows) per call
            nf = io.tile([P, NT * WROW], f16, tag="nf")
            if do_gather:
                if gather_mode == "block":
                    nc.gpsimd.indirect_dma_start(
                        out=nf[:],
                        out_offset=None,
                        in_=scomb_d.ap(),
                        in_offset=bass.IndirectOffsetOnAxis(ap=inds[:, :], axis=0),
                    )
                else:
                    for t in range(ntb):
                        nc.gpsimd.indirect_dma_start(
                            out=nf[:, t * WROW : (t + 1) * WROW],
                            out_offset=None,
                            in_=scomb_d.ap(),
                            in_offset=bass.IndirectOffsetOnAxis(
                                ap=inds[:, t : t + 1], axis=0
                            ),
                        )
            else:
                nc.gpsimd.memset(nf[:], 0.5)
            if not do_compute:
                tsel = NT - 1 if parts == "gather_t" else 0
                osb0 = mid.tile([P, C], f32, tag="osb")
                nc.vector.tensor_copy(
                    osb0[:], nf[:, tsel * WROW + CO : tsel * WROW + CO + C]
                )
                nc.sync.dma_start(out_d.ap()[B], osb0[:])
                continue

            nfv = nf[:].rearrange("p (t w) -> p t w", w=WROW)

            if parts == "coords4":
                osb0 = mid.tile([P, C], f32, tag="osb")
                nc.vector.tensor_copy(
                    osb0[:].rearrange("p (t j) -> p t j", j=4), nfv[:, :, 0:4]
                )
                nc.sync.dma_start(out_d.ap()[B], osb0[:])
                continue

            if parts in ("coords", "qbp"):
                osb0 = mid.tile([P, C], f32, tag="osb")
                nc.gpsimd.memset(osb0[:], 0.0)
                src = (
                    nfv[:, :, 0:3]
                    if parts == "coords"
                    else qb[:].rearrange("p (t j) -> p t j", j=3)
                )
                nc.vector.tensor_copy(
                    osb0[:, : NT * 3].rearrange("p (t j) -> p t j", j=3), src
                )
                nc.sync.dma_start(out_d.ap()[B], osb0[:])
                continue

            # influence + step A over the ntb valid tiles. The tail block
            # runs in two PSUM-bank-aligned halves so the first half's
            # compute overlaps the second half's gathers, shortening the
            # end-of-pipeline drain after the final gather.
            delta = mid.tile([P, NT * 3], f16, tag="delta")
            diff = mid.tile([P, NT * K * 3], f16, tag="diff")
            sq = mid.tile([P, NT * K * 3], f16, tag="sq")
            d2 = mid.tile([P, NT * K], f16, tag="d2")
            dd = mid.tile([P, NT * K], f16, tag="dd")
            infl = mid.tile([P, NT * K], f16, tag="infl")
            bd = mid.tile([P, NT * G * K], f16, tag="bd")
            # step A output: wfT[c, m*15+k] in 4 PSUM banks. Tail block:
            # unwritten PSUM cols hold the previous block's (finite)
            # values; they only reach discarded padding rows.
            pa = [
                psa.tile([P, 8 * G * K], f32, tag=f"psA{q}", name=f"psA{q}")
                for q in range(4)
            ]
            wfT = mid.tile([P, P * K], f16, tag="wfT")

            ranges = [(0, ntb)] if full else [(0, 16), (16, 24), (24, ntb)]
            for t0, t1 in ranges:
                nt_r = t1 - t0
                nc.vector.tensor_tensor(
                    delta[:, t0 * 3 : t1 * 3].rearrange(
                        "p (t j) -> p t j", j=3
                    ),
                    nfv[:, t0:t1, 0:3],
                    qb[:, t0 * 3 : t1 * 3].rearrange("p (t j) -> p t j", j=3),
                    op=sub,
                )
                nc.vector.tensor_tensor(
                    diff[:, t0 * K * 3 : t1 * K * 3].rearrange(
                        "p (t k j) -> p t k j", k=K, j=3
                    ),
                    delta[:, t0 * 3 : t1 * 3]
                    .rearrange("p (t j) -> p t j", j=3)
                    .unsqueeze(2)
                    .broadcast_to([P, nt_r, K, 3]),
                    kp_sb[:].rearrange("p (k j) -> p k j", j=3)
                    .unsqueeze(1)
                    .broadcast_to([P, nt_r, K, 3]),
                    op=sub,
                )
                nc.vector.tensor_tensor(
                    sq[:, t0 * K * 3 : t1 * K * 3],
                    diff[:, t0 * K * 3 : t1 * K * 3],
                    diff[:, t0 * K * 3 : t1 * K * 3],
                    op=mult,
                )
                with nc.allow_low_precision(reason="sum of 3 fp16 squares"):
                    nc.vector.reduce_sum(
                        out=d2[:, t0 * K : t1 * K],
                        in_=sq[:, t0 * K * 3 : t1 * K * 3].rearrange(
                            "p (tk j) -> p tk j", j=3
                        ),
                        axis=mybir.AxisListType.X,
                    )
                nc.scalar.sqrt(dd[:, t0 * K : t1 * K], d2[:, t0 * K : t1 * K])
                nc.scalar.activation(
                    infl[:, t0 * K : t1 * K],
                    dd[:, t0 * K : t1 * K],
                    mybir.ActivationFunctionType.Relu,
                    bias=1.0,
                    scale=-1.0 / SIGMA,
                )
                # block-diagonal influence [p, t*60 + g*15 + k]
                nc.vector.tensor_tensor(
                    bd[:, t0 * G * K : t1 * G * K].rearrange(
                        "p (t g k) -> p t g k", g=G, k=K
                    ),
                    infl[:, t0 * K : t1 * K]
                    .rearrange("p (t k) -> p t k", k=K)
                    .unsqueeze(2)
                    .broadcast_to([P, nt_r, G, K]),
                    mask_sb[:].rearrange("p (g k) -> p g k", k=K)
                    .unsqueeze(1)
                    .broadcast_to([P, nt_r, G, K]),
                    op=mult,
                )
                for t in range(t0, t1):
                    nc.tensor.matmul(
                        pa[t // 8][
                            :, (t % 8) * (G * K) : (t % 8 + 1) * (G * K)
                        ],
                        lhsT=nfv[:, t, CO : CO + C],
                        rhs=bd[:, t * (G * K) : (t + 1) * (G * K)],
                        start=True,
                        stop=True,
                    )
                for q in range(t0 // 8, (t1 + 7) // 8):
                    hi = min(480, (t1 - q * 8) * G * K)
                    nc.scalar.copy(
                        wfT[:, q * 480 : q * 480 + hi], pa[q][:, :hi]
                    )

            # step B: accumulate over k
            outp = psb.tile([P, C], f32, tag="outp")
            wview = wfT[:].rearrange("p (m k) -> p k m", k=K)
            for k in range(K):
                nc.tensor.matmul(
                    outp[:],
                    lhsT=wview[:, k, :],
                    rhs=w_sb[:, k, :],
                    start=(k == 0),
                    stop=(k == K - 1),
                )
            osb = mid.tile([P, C], f32, tag="osb")
            nc.scalar.copy(osb[:], outp[:])
            nc.sync.dma_start(out_d.ap()[B], osb[:])

    if compile:
        nc.compile()
    return nc


def _host_prep(q_pts, s_pts, s_feats, neighb_inds, weights, kernel_points):
    """Shard + lay out inputs for the 8 cores."""
    q_pts = np.asarray(q_pts, np.float32)
    s_pts = np.asarray(s_pts, np.float32)
    s_feats = np.asarray(s_feats, np.float32)
    neighb_inds = np.asarray(neighb_inds, np.int32)
    weights = np.asarray(weights, np.float16)
    kernel_points = np.asarray(kernel_points, np.float32)

    s_comb = np.zeros((len(s_pts), WROW), np.float16)
    s_comb[:, 0:3] = s_pts.astype(np.float16)
    s_comb[:, CO : CO + C] = s_feats.astype(np.float16)

    kp_rep = np.broadcast_to(
        kernel_points.astype(np.float16).reshape(1, K * 3), (P, K * 3)
    ).copy()
    mask60 = (
        (np.arange(G * K)[None, :] // K) == (np.arange(P)[:, None] // H)
    ).astype(np.float16)

    in_maps = []
    for i in range(N_CORES):
        sl = slice(i * M_CORE, (i + 1) * M_CORE)
        q = np.zeros((M_PAD, 3), np.float32)
        q[:M_CORE] = q_pts[sl]
        idx = np.zeros((M_PAD, H), np.int32)
        idx[:M_CORE] = neighb_inds[sl]

        # inds_blk[B, g*32+h, t] = idx[B*128 + 4t + g, h]
        a = idx.reshape(NB, NT, G, H)            # [B, t, g, h]
        inds_blk = np.ascontiguousarray(
            a.transpose(0, 2, 3, 1)              # [B, g, h, t]
        ).reshape(NB, P, NT)

        # q_blk[B, g*32+h, 3t+j] = q[B*128 + 4t + g, j]
        b = q.reshape(NB, NT, G, 3)              # [B, t, g, j]
        b = b.transpose(0, 2, 1, 3)              # [B, g, t, j]
        q_blk = np.repeat(
            b.reshape(NB, G, 1, NT * 3), H, axis=2
        ).reshape(NB, P, NT * 3)

        in_maps.append(
            {
                "q_blk": q_blk.astype(np.float16),
                "inds_blk": inds_blk,
                "s_comb": s_comb,
                "weights": weights,
                "kp_rep": kp_rep,
                "mask60": mask60,
            }
        )
    return in_maps


def kernel(q_pts, s_pts, s_feats, neighb_inds, weights, kernel_points):
    global _compiled
    if _compiled is None:
        _compiled = _build_bass()
    nc = _compiled

    from concourse.bass_utils import run_bass_kernel_spmd

    in_maps = _host_prep(
        q_pts, s_pts, s_feats, neighb_inds, weights, kernel_points
    )
    res = run_bass_kernel_spmd(nc, in_maps, core_ids=list(range(N_CORES)))
    out = np.concatenate(
        [r["out"].reshape(M_PAD, C)[:M_CORE] for r in res.results], axis=0
    )
    return out.astype(np.float32)


if __name__ == "__main__":
    rng = np.random.default_rng(0)
    ins = {
        "q_pts": rng.standard_normal((M_TOTAL, 3)).astype(np.float32),
        "s_pts": rng.standard_normal((N_SUP, 3)).astype(np.float32),
        "s_feats": rng.standard_normal((N_SUP, C)).astype(np.float32),
        "neighb_inds": rng.integers(0, N_SUP, (M_TOTAL, H)).astype(np.int32),
        "weights": (rng.standard_normal((K, C, C)) * 0.05).astype(np.float32),
        "kernel_points": rng.standard_normal((K, 3)).astype(np.float32),
    }
    out = kernel(**ins)
    print(out.shape, out.dtype)
